# revision 31
# baseline (speedup 1.0000x reference)
"""Multi-head attention (B=2, T=2048, C=1024, H=16) on 8 trn2 cores.

Sharding: core c -> batch b = c//4, head-group g = c%4 (4 heads, proj cols
[g*256, (g+1)*256)).  Host pre-transposes per-batch inputs to feature-major
[C, T] so every device matmul has its contraction dim on SBUF partitions.
Each core computes a partial output  O_g @ Wo[g-rows]  [2048, 1024]; the
host sums the 4 partials per batch and adds bo.

v3 (vs the original): the out-proj contracts K=128 by stacking head pairs
on partitions (normalize writes head 2h+1 to partitions 64-127), halving
out-proj tensor time; the denominator broadcast matmuls are replaced by
DVE reciprocal + GpSimd partition_broadcast; Q/K projection evacuation
moved to the ACT engine (Identity+bias) to keep DVE headroom.  All value
tensors stay bf16 -- fp8 was measured to cost ~10% output error (attention
averaging passes per-element relative noise straight through).
"""

import ml_dtypes
import numpy as np


import concourse.bass as bass
import concourse.tile as tile
from concourse import bacc, mybir
from concourse.bass_utils import run_bass_kernel_spmd

B, T, C, H, D = 2, 2048, 1024, 16, 64
N_CORES = 8
GROUPS = 4          # head-groups (cores per batch)
HG = H // GROUPS    # heads per core = 4
CG = HG * D         # proj cols per core = 256
KT = C // 128       # contraction k-tiles = 8
SCALE = D ** -0.5   # 1/8

F32 = mybir.dt.float32
BF16 = mybir.dt.bfloat16
I16 = mybir.dt.int16
AF = mybir.ActivationFunctionType
ALU = mybir.AluOpType

# Schraudolph exp for the DVE: bf16 bitpattern = round(A16*s + B16) approximates
# exp(s/8).  Every other exp tile runs here: the serialized ACT exp chain
# (2x1105ns per 2 tk through the sa/sb double-buffer) is the steady-state
# cadence governor, and alternating engines pipelines the two exps.  The
# ±3% sawtooth on half the weights adds ~1.2% sigma -> ~1.6e-2 total rel err.
LOG2E = 1.4426950408889634
A16 = 128 * 0.125 * LOG2E
B16 = 128 * (127 - 0.0579)
EXP_DVE_EVERY = 2


def build_mha_program():
    """Build the SPMD Bass program (identical on all 8 cores)."""
    nc = bacc.Bacc("TRN2", target_bir_lowering=False, debug=False,
                   num_devices=N_CORES)

    xqT = nc.dram_tensor("xqT", (C, T), BF16, kind="ExternalInput").ap()
    xkT = nc.dram_tensor("xkT", (C, T), BF16, kind="ExternalInput").ap()
    xvT = nc.dram_tensor("xvT", (C, T), BF16, kind="ExternalInput").ap()
    wq = nc.dram_tensor("wq", (C, CG), BF16, kind="ExternalInput").ap()
    wk = nc.dram_tensor("wk", (C, CG), BF16, kind="ExternalInput").ap()
    wv = nc.dram_tensor("wv", (C, CG), BF16, kind="ExternalInput").ap()
    bq = nc.dram_tensor("bq", (CG,), F32, kind="ExternalInput").ap()
    bk = nc.dram_tensor("bk", (CG,), F32, kind="ExternalInput").ap()
    bv = nc.dram_tensor("bv", (CG,), F32, kind="ExternalInput").ap()
    wo = nc.dram_tensor("wo", (CG, C), BF16, kind="ExternalInput").ap()
    yp = nc.dram_tensor("yp", (T, C), F32, kind="ExternalOutput").ap()

    with tile.TileContext(nc) as tc:
        _emit(tc, xqT, xkT, xvT, wq, wk, wv, bq, bk, bv, wo, yp)
    nc.compile()
    return nc


def _emit(tc, xqT, xkT, xvT, wq, wk, wv, bq, bk, bv, wo, yp):
    nc = tc.nc
    MT = CG // 128            # stationary tiles per projection = 2
    MC = 512                  # chunk width (tokens) everywhere
    NMC = T // MC             # 4 chunks
    TT = T // 128             # 16 t-tiles
    QC = 512                  # q-chunk width in attention
    NQC = T // QC             # 4 q-chunks
    VS = D + 1                # 65: V cols + ones col per head

    from contextlib import ExitStack
    with ExitStack() as ctx:
        consts = ctx.enter_context(tc.tile_pool(name="consts", bufs=1))
        xs_pool = ctx.enter_context(tc.tile_pool(name="xs", bufs=12))
        big = ctx.enter_context(tc.tile_pool(name="big", bufs=1))
        e_pool = ctx.enter_context(tc.tile_pool(name="e", bufs=6))
        ev_pool = ctx.enter_context(tc.tile_pool(name="ev", bufs=3))
        nrm_pool = ctx.enter_context(tc.tile_pool(name="nrm", bufs=4))
        pp = ctx.enter_context(tc.tile_pool(name="pp", bufs=2, space="PSUM"))
        pv_ps = ctx.enter_context(tc.tile_pool(name="pvps", bufs=2, space="PSUM"))
        sa_ps = ctx.enter_context(tc.tile_pool(name="saps", bufs=1, space="PSUM"))
        sb_ps = ctx.enter_context(tc.tile_pool(name="sbps", bufs=1, space="PSUM"))

        # Per-chunk persistent activations: fine-grained tiles so stages
        # pipeline at chunk granularity instead of a hard phase boundary.
        qTc = [big.tile([128, MT, MC], BF16, name=f"qTc{i}", tag=f"qTc{i}")
               for i in range(NMC)]
        kTc = [big.tile([128, MT, MC], BF16, name=f"kTc{i}", tag=f"kTc{i}")
               for i in range(NMC)]
        vc = [big.tile([128, MC // 128, HG * VS], BF16, name=f"vc{i}",
                       tag=f"vc{i}") for i in range(NMC)]
        # out-proj stationary: head pair g2 stacked on partitions
        # (head 2*g2 -> partitions 0-63, head 2*g2+1 -> partitions 64-127)
        oc = [big.tile([128, HG // 2, QC], BF16, name=f"oc{i}", tag=f"oc{i}")
              for i in range(NQC)]

        wq_s = consts.tile([128, KT, CG], BF16, tag="wq")
        wk_s = consts.tile([128, KT, CG], BF16, tag="wk")
        wv_s = consts.tile([128, KT, CG], BF16, tag="wv")
        wo_s = consts.tile([128, HG // 2, C], BF16, tag="wo")
        bq_s = consts.tile([128, MT, 1], F32, tag="bq")
        bk_s = consts.tile([128, MT, 1], F32, tag="bk")
        bv_bc = consts.tile([128, CG], F32, tag="bv")

        def load_x(src, name):
            x_t = xs_pool.tile([128, KT, MC], BF16, tag="xs", name=name)
            nc.sync.dma_start(out=x_t,
                              in_=src.rearrange("(kt p) m -> p kt m", p=128))
            return x_t

        def emit_a(mc, xq_t, xk_t):
            """Q/K projection for chunk mc; yields every 4 matmuls."""
            for x_t, w_s, b_s, dstl in ((xq_t, wq_s, bq_s, qTc),
                                        (xk_t, wk_s, bk_s, kTc)):
                for mt in range(MT):
                    ps = pp.tile([128, 512], F32, tag="pp")
                    for kt in range(KT):
                        nc.tensor.matmul(
                            ps[:, :MC],
                            w_s[:, kt, bass.ts(mt, 128)],
                            x_t[:, kt, :],
                            start=(kt == 0), stop=(kt == KT - 1))
                        if kt == 3:
                            yield
                    nc.vector.tensor_scalar_add(
                        dstl[mc][:, mt, :], ps[:, :MC], b_s[:, mt, :])
                    yield

        def emit_b(mc, xv_t):
            """V projection for chunk mc; yields every 8 matmuls (2 subs)."""
            v4 = vc[mc].rearrange("p t (h c) -> p t h c", h=HG)
            nc.vector.memset(v4[:, :, :, D:VS], 1.0)
            for sub in range(MC // 128):
                ps = pp.tile([128, 512], F32, tag="pp")
                for kt in range(KT):
                    nc.tensor.matmul(
                        ps[:, :CG],
                        xv_t[:, kt, bass.ts(sub, 128)],
                        wv_s[:, kt, :],
                        start=(kt == 0), stop=(kt == KT - 1))
                    if kt == 3:
                        yield
                nc.vector.tensor_add(
                    v4[:, sub, :, 0:D],
                    ps[:, :CG].rearrange("p (h c) -> p h c", h=HG),
                    bv_bc.rearrange("p (h c) -> p h c", h=HG))
                yield

        def emit_d_unit(qc, tl, cc):
            trows_out = bass.ts(qc * (QC // 128) + tl, 128)
            ps = pp.tile([128, 512], F32, tag="pp")
            for g2 in range(HG // 2):
                nc.tensor.matmul(
                    ps,
                    oc[qc][:, g2, bass.ts(tl, 128)],
                    wo_s[:, g2, bass.ts(cc, 512)],
                    start=(g2 == 0), stop=(g2 == HG // 2 - 1))
            ev = ev_pool.tile([128, 512], F32, tag="ev")
            nc.scalar.copy(ev, ps)
            nc.sync.dma_start(out=yp[trows_out, bass.ts(cc, 512)], in_=ev)

        def c_iter(qc, hp):
            """Attention for head pair hp on q-chunk qc.  Yields after every
            tk so filler matmuls can interleave at fine granularity."""
            po = [pv_ps.tile([128, 512], F32, tag="pv",
                             name=f"po{qc}_{hp}_{i}") for i in range(2)]
            def emit_pv(e_prev, tkp):
                for h01 in range(2):
                    nc.tensor.matmul(
                        po[h01][0:VS, :],
                        vc[tkp // 4][:, tkp % 4,
                                     bass.ds((2 * hp + h01) * VS, VS)],
                        e_prev[:, h01, :],
                        start=(tkp == 0), stop=(tkp == TT - 1))

            pending = None    # issue PV one tk late so its exp-wait is
            for tk in range(TT):   # already satisfied at the queue head
                pool = sa_ps if tk % 2 == 0 else sb_ps
                ps = pool.tile([128, 2, 512], F32,
                               tag="sa" if tk % 2 == 0 else "sb")
                for h01 in range(2):
                    pb = h01 * D
                    nc.tensor.matmul(
                        ps[:, h01, :],
                        kTc[tk // 4][pb:pb + D, hp, bass.ts(tk % 4, 128)],
                        qTc[qc][pb:pb + D, hp, :],
                        start=True, stop=True)
                e_t = e_pool.tile([128, 2, 512], BF16, tag="e")
                if tk % EXP_DVE_EVERY == EXP_DVE_EVERY - 1:
                    nc.vector.tensor_scalar(e_t.bitcast(I16), ps,
                                            A16, B16, ALU.mult, ALU.add)
                else:
                    nc.scalar.activation(e_t, ps, AF.Exp, scale=SCALE)
                if pending is not None:
                    emit_pv(*pending)
                pending = (e_t, tk)
                yield
            emit_pv(*pending)
            yield
            # Evacuate po to SBUF first (frees the pv psum tiles for the
            # next c_iter's PV almost immediately); normalize from the copy.
            pocs = []
            for h01 in range(2):
                poc = nrm_pool.tile([128, 512], F32, tag="poc",
                                    name=f"poc{qc}_{hp}_{h01}")
                nc.vector.tensor_copy(poc[0:VS, :], po[h01][0:VS, :])
                pocs.append(poc)
            for h01 in range(2):
                poc = pocs[h01]
                den_s = nrm_pool.tile([1, 2, 512], F32, tag="dens")
                nc.vector.tensor_mul(den_s[:, 0, :], poc[D:D + 1, :],
                                     ones_r[D:D + 1, :])
                nc.vector.reciprocal_approx_fast(den_s[:, 1, :],
                                                 den_s[:, 0, :])
                rec_b = nrm_pool.tile([64, 512], F32, tag="recb")
                nc.gpsimd.partition_broadcast(rec_b, den_s[:, 1, :])
                nc.vector.tensor_mul(
                    oc[qc][64 * h01:64 * h01 + 64, hp, :],
                    poc[0:D, :], rec_b)

        # ---- emission schedule (software pipeline) --------------------
        # Critical-path DMAs first (Q-proj inputs), everything prefetched up
        # front so proj matmuls never head-of-line block the tensor queue.
        nc.sync.dma_start(out=wq_s, in_=wq.rearrange("(kt p) c -> p kt c", p=128))
        xq_ts = {0: load_x(xqT[:, bass.ts(0, MC)], "xq0")}
        nc.sync.dma_start(out=wk_s, in_=wk.rearrange("(kt p) c -> p kt c", p=128))
        xk_ts = {0: load_x(xkT[:, bass.ts(0, MC)], "xk0")}
        nc.sync.dma_start(
            out=bq_s, in_=bq.rearrange("(mt p) -> p mt", p=128).unsqueeze(2))
        nc.sync.dma_start(
            out=bk_s, in_=bk.rearrange("(mt p) -> p mt", p=128).unsqueeze(2))
        nc.sync.dma_start(out=wv_s, in_=wv.rearrange("(kt p) c -> p kt c", p=128))
        nc.sync.dma_start(
            out=bv_bc,
            in_=bass.AP(tensor=bv.tensor, offset=bv.offset,
                        ap=[[0, 128]] + list(bv.ap)))
        xv_ts = {0: load_x(xvT[:, bass.ts(0, MC)], "xv0")}
        # touch Exp once so the ACT table loads during the DMA-bound head
        warm = consts.tile([128, 1], F32, tag="warm")
        nc.vector.memset(warm[0:1, :], 0.0)
        nc.scalar.activation(warm[0:1, :], warm[0:1, :], AF.Exp)
        ones_r = consts.tile([128, 512], F32, tag="onesr")
        nc.vector.memset(ones_r[D:D + 1, :], 1.0)
        # PE warm-up: dummy matmuls during the DMA head ramp the p-state
        # (PE reaches max clock after ~3us of continuous execution).
        wx = consts.tile([128, 512], BF16, tag="wx")
        nc.vector.memset(wx[:, 0:8], 0.0)
        psd = pp.tile([128, 512], F32, tag="pp", name="warmps")
        for i in range(16):
            nc.tensor.matmul(psd[0:8, :], wx[:, 0:8], wx,
                             start=(i == 0), stop=(i == 15))
        for mc in range(1, NMC):
            xq_ts[mc] = load_x(xqT[:, bass.ts(mc, MC)], f"xq{mc}")
            xk_ts[mc] = load_x(xkT[:, bass.ts(mc, MC)], f"xk{mc}")
            xv_ts[mc] = load_x(xvT[:, bass.ts(mc, MC)], f"xv{mc}")
        nc.sync.dma_start(out=wo_s, in_=wo.rearrange("(g p) c -> p g c", p=128))

        # chunk 0 proj fully, before attention can start
        for _ in emit_a(0, xq_ts[0], xk_ts[0]):
            pass
        for _ in emit_b(0, xv_ts[0]):
            pass

        # filler stream: proj chunks 1-3 (quanta), then out-proj d-units
        def filler_stream():
            for mc in range(1, NMC):
                yield from emit_a(mc, xq_ts[mc], xk_ts[mc])
                yield from emit_b(mc, xv_ts[mc])

        fillers = filler_stream()
        proj_left = 3 * 16           # filler quanta in proj phase
        d_queue = []                 # (qc, tl, cc) units, 2 MMs each

        def pump(n):
            nonlocal proj_left
            for _ in range(n):
                if next(fillers, StopIteration) is not StopIteration:
                    proj_left -= 1
                elif d_queue:
                    emit_d_unit(*d_queue.pop(0))
                else:
                    break

        prev_gen = None
        for qc in range(NQC):
            for hp in range(HG // 2):
                gen = c_iter(qc, hp)
                if prev_gen is not None:
                    for _ in prev_gen:       # previous iteration's norm
                        pass
                    prev_gen = None
                    if hp == 0 and qc > 0:   # qc-1 fully normalized now
                        d_queue += [(qc - 1, tl, cc)
                                    for tl in range(4) for cc in range(2)]
                for _ in range(TT):
                    next(gen)
                    # proj phase: 4 quanta/tk keeps chunk mc fully emitted
                    # before attention reaches tk=4*mc (in-order queues would
                    # otherwise head-of-line deadlock); then 1 d-unit/tk.
                    pump(4 if proj_left > 0 else 1)
                prev_gen = gen
        for _ in prev_gen:
            pass
        pump(10**6)
        d_queue += [(NQC - 1, tl, cc) for tl in range(4) for cc in range(2)]
        for u in d_queue:
            emit_d_unit(*u)


_NC_CACHE = None


def _get_program():
    global _NC_CACHE
    if _NC_CACHE is None:
        _NC_CACHE = build_mha_program()
    return _NC_CACHE


def make_in_maps(query, key, value, Wq, bq, Wk, bk, Wv, bv, Wo):
    q = np.asarray(query, np.float32).reshape(B, T, C)
    k = np.asarray(key, np.float32).reshape(B, T, C)
    v = np.asarray(value, np.float32).reshape(B, T, C)
    xT = {n: [np.ascontiguousarray(a[b].T).astype(ml_dtypes.bfloat16)
              for b in range(B)]
          for n, a in (("q", q), ("k", k), ("v", v))}
    in_maps = []
    for c in range(N_CORES):
        b, g = divmod(c, GROUPS)
        sl = slice(g * CG, (g + 1) * CG)
        in_maps.append({
            "xqT": xT["q"][b], "xkT": xT["k"][b], "xvT": xT["v"][b],
            "wq": np.ascontiguousarray(np.asarray(Wq, np.float32)[:, sl]).astype(ml_dtypes.bfloat16),
            "wk": np.ascontiguousarray(np.asarray(Wk, np.float32)[:, sl]).astype(ml_dtypes.bfloat16),
            "wv": np.ascontiguousarray(np.asarray(Wv, np.float32)[:, sl]).astype(ml_dtypes.bfloat16),
            "bq": np.ascontiguousarray(np.asarray(bq, np.float32)[sl]),
            "bk": np.ascontiguousarray(np.asarray(bk, np.float32)[sl]),
            "bv": np.ascontiguousarray(np.asarray(bv, np.float32)[sl]),
            "wo": np.ascontiguousarray(np.asarray(Wo, np.float32)[sl, :]).astype(ml_dtypes.bfloat16),
        })
    return in_maps


def assemble_output(results, bo):
    y = np.zeros((B, T, C), np.float32)
    for c, res in enumerate(results):
        y[c // GROUPS] += res["yp"]
    y += np.asarray(bo, np.float32)
    return y


def kernel(query, key, value, Wq, bq, Wk, bk, Wv, bv, Wo, bo):
    nc = _get_program()
    in_maps = make_in_maps(query, key, value, Wq, bq, Wk, bk, Wv, bv, Wo)
    res = run_bass_kernel_spmd(nc, in_maps, list(range(N_CORES)))
    return assemble_output(res.results, bo)


# revision 34
# speedup vs baseline: 1.1110x; 1.1110x over previous
"""Multi-head attention (B=2, T=2048, C=1024, H=16) on 8 trn2 cores.

Sharding: core c -> batch b = c//4, head-group g = c%4 (4 heads, proj cols
[g*256, (g+1)*256)).  Host pre-transposes per-batch inputs to feature-major
[C, T] so every device matmul has its contraction dim on SBUF partitions.
Each core computes a partial output  O_g @ Wo[g-rows]  [2048, 1024]; the
host sums the 4 partials per batch and adds bo.

v3 (vs the original): the out-proj contracts K=128 by stacking head pairs
on partitions (normalize writes head 2h+1 to partitions 64-127), halving
out-proj tensor time; the denominator broadcast matmuls are replaced by
DVE reciprocal + GpSimd partition_broadcast; Q/K projection evacuation
moved to the ACT engine (Identity+bias) to keep DVE headroom.  All value
tensors stay bf16 -- fp8 was measured to cost ~10% output error (attention
averaging passes per-element relative noise straight through).
"""

import ml_dtypes
import numpy as np


import concourse.bass as bass
import concourse.tile as tile
from concourse import bacc, mybir
from concourse.bass_utils import run_bass_kernel_spmd

B, T, C, H, D = 2, 2048, 1024, 16, 64
N_CORES = 8
GROUPS = 4          # head-groups (cores per batch)
HG = H // GROUPS    # heads per core = 4
CG = HG * D         # proj cols per core = 256
KT = C // 128       # contraction k-tiles = 8
SCALE = D ** -0.5   # 1/8

F32 = mybir.dt.float32
BF16 = mybir.dt.bfloat16
I16 = mybir.dt.int16
AF = mybir.ActivationFunctionType
ALU = mybir.AluOpType

# Schraudolph exp for the DVE: bf16 bitpattern = round(A16*s + B16) approximates
# exp(s/8).  Every other exp tile runs here: the serialized ACT exp chain
# (2x1105ns per 2 tk through the sa/sb double-buffer) is the steady-state
# cadence governor, and alternating engines pipelines the two exps.  The
# ±3% sawtooth on half the weights adds ~1.2% sigma -> ~1.6e-2 total rel err.
LOG2E = 1.4426950408889634
A16 = 128 * 0.125 * LOG2E
B16 = 128 * (127 - 0.0579)
EXP_DVE_EVERY = 2


def build_mha_program():
    """Build the SPMD Bass program (identical on all 8 cores)."""
    nc = bacc.Bacc("TRN2", target_bir_lowering=False, debug=False,
                   num_devices=N_CORES)

    xqT = nc.dram_tensor("xqT", (C, T), BF16, kind="ExternalInput").ap()
    xkT = nc.dram_tensor("xkT", (C, T), BF16, kind="ExternalInput").ap()
    xvT = nc.dram_tensor("xvT", (C, T), BF16, kind="ExternalInput").ap()
    wq = nc.dram_tensor("wq", (C, CG), BF16, kind="ExternalInput").ap()
    wk = nc.dram_tensor("wk", (C, CG), BF16, kind="ExternalInput").ap()
    wv = nc.dram_tensor("wv", (C, CG), BF16, kind="ExternalInput").ap()
    bq = nc.dram_tensor("bq", (CG,), F32, kind="ExternalInput").ap()
    bk = nc.dram_tensor("bk", (CG,), F32, kind="ExternalInput").ap()
    bv = nc.dram_tensor("bv", (CG,), F32, kind="ExternalInput").ap()
    wo = nc.dram_tensor("wo", (CG, C), BF16, kind="ExternalInput").ap()
    yp = nc.dram_tensor("yp", (T, C), F32, kind="ExternalOutput").ap()

    with tile.TileContext(nc) as tc:
        _emit(tc, xqT, xkT, xvT, wq, wk, wv, bq, bk, bv, wo, yp)
    nc.compile()
    return nc


def _emit(tc, xqT, xkT, xvT, wq, wk, wv, bq, bk, bv, wo, yp):
    nc = tc.nc
    MT = CG // 128            # stationary tiles per projection = 2
    MC = 512                  # chunk width (tokens) everywhere
    NMC = T // MC             # 4 chunks
    TT = T // 128             # 16 t-tiles
    QC = 512                  # q-chunk width in attention
    NQC = T // QC             # 4 q-chunks
    VS = D + 1                # 65: V cols + ones col per head

    norm_q = []   # staged normalization closures (pumped 1 per tk)

    from contextlib import ExitStack
    with ExitStack() as ctx:
        consts = ctx.enter_context(tc.tile_pool(name="consts", bufs=1))
        xs_pool = ctx.enter_context(tc.tile_pool(name="xs", bufs=12))
        big = ctx.enter_context(tc.tile_pool(name="big", bufs=1))
        e_pool = ctx.enter_context(tc.tile_pool(name="e", bufs=6))
        ev_pool = ctx.enter_context(tc.tile_pool(name="ev", bufs=3))
        nrm_pool = ctx.enter_context(tc.tile_pool(name="nrm", bufs=4))
        pp = ctx.enter_context(tc.tile_pool(name="pp", bufs=2, space="PSUM"))
        pv_ps = ctx.enter_context(tc.tile_pool(name="pvps", bufs=2, space="PSUM"))
        sa_ps = ctx.enter_context(tc.tile_pool(name="saps", bufs=1, space="PSUM"))
        sb_ps = ctx.enter_context(tc.tile_pool(name="sbps", bufs=1, space="PSUM"))

        # Per-chunk persistent activations: fine-grained tiles so stages
        # pipeline at chunk granularity instead of a hard phase boundary.
        qTc = [big.tile([128, MT, MC], BF16, name=f"qTc{i}", tag=f"qTc{i}")
               for i in range(NMC)]
        kTc = [big.tile([128, MT, MC], BF16, name=f"kTc{i}", tag=f"kTc{i}")
               for i in range(NMC)]
        vc = [big.tile([128, MC // 128, HG * VS], BF16, name=f"vc{i}",
                       tag=f"vc{i}") for i in range(NMC)]
        # out-proj stationary: head pair g2 stacked on partitions
        # (head 2*g2 -> partitions 0-63, head 2*g2+1 -> partitions 64-127)
        oc = [big.tile([128, HG // 2, QC], BF16, name=f"oc{i}", tag=f"oc{i}")
              for i in range(NQC)]

        wq_s = consts.tile([128, KT, CG], BF16, tag="wq")
        wk_s = consts.tile([128, KT, CG], BF16, tag="wk")
        wv_s = consts.tile([128, KT, CG], BF16, tag="wv")
        wo_s = consts.tile([128, HG // 2, C], BF16, tag="wo")
        bq_s = consts.tile([128, MT, 1], F32, tag="bq")
        bk_s = consts.tile([128, MT, 1], F32, tag="bk")
        bv_bc = consts.tile([128, CG], F32, tag="bv")

        def load_x(src, name):
            x_t = xs_pool.tile([128, KT, MC], BF16, tag="xs", name=name)
            nc.sync.dma_start(out=x_t,
                              in_=src.rearrange("(kt p) m -> p kt m", p=128))
            return x_t

        def emit_a(mc, xq_t, xk_t):
            """Q/K projection for chunk mc; yields every 4 matmuls."""
            for x_t, w_s, b_s, dstl in ((xq_t, wq_s, bq_s, qTc),
                                        (xk_t, wk_s, bk_s, kTc)):
                for mt in range(MT):
                    ps = pp.tile([128, 512], F32, tag="pp")
                    for kt in range(KT):
                        nc.tensor.matmul(
                            ps[:, :MC],
                            w_s[:, kt, bass.ts(mt, 128)],
                            x_t[:, kt, :],
                            start=(kt == 0), stop=(kt == KT - 1))
                        if kt == 3:
                            yield
                    nc.vector.tensor_scalar_add(
                        dstl[mc][:, mt, :], ps[:, :MC], b_s[:, mt, :])
                    yield

        def emit_b(mc, xv_t):
            """V projection for chunk mc; yields every 8 matmuls (2 subs)."""
            v4 = vc[mc].rearrange("p t (h c) -> p t h c", h=HG)
            nc.vector.memset(v4[:, :, :, D:VS], 1.0)
            for sub in range(MC // 128):
                ps = pp.tile([128, 512], F32, tag="pp")
                for kt in range(KT):
                    nc.tensor.matmul(
                        ps[:, :CG],
                        xv_t[:, kt, bass.ts(sub, 128)],
                        wv_s[:, kt, :],
                        start=(kt == 0), stop=(kt == KT - 1))
                    if kt == 3:
                        yield
                nc.vector.tensor_add(
                    v4[:, sub, :, 0:D],
                    ps[:, :CG].rearrange("p (h c) -> p h c", h=HG),
                    bv_bc.rearrange("p (h c) -> p h c", h=HG))
                yield

        def emit_d_unit(qc, tl, cc):
            trows_out = bass.ts(qc * (QC // 128) + tl, 128)
            ps = pp.tile([128, 512], F32, tag="pp")
            for g2 in range(HG // 2):
                nc.tensor.matmul(
                    ps,
                    oc[qc][:, g2, bass.ts(tl, 128)],
                    wo_s[:, g2, bass.ts(cc, 512)],
                    start=(g2 == 0), stop=(g2 == HG // 2 - 1))
            ev = ev_pool.tile([128, 512], F32, tag="ev")
            nc.scalar.copy(ev, ps)
            nc.sync.dma_start(out=yp[trows_out, bass.ts(cc, 512)], in_=ev)

        def c_iter(qc, hp):
            """Attention for head pair hp on q-chunk qc.  Yields after every
            tk so filler matmuls can interleave at fine granularity."""
            po = [pv_ps.tile([128, 512], F32, tag="pv",
                             name=f"po{qc}_{hp}_{i}") for i in range(2)]
            def emit_pv(e_prev, tkp):
                for h01 in range(2):
                    nc.tensor.matmul(
                        po[h01][0:VS, :],
                        vc[tkp // 4][:, tkp % 4,
                                     bass.ds((2 * hp + h01) * VS, VS)],
                        e_prev[:, h01, :],
                        start=(tkp == 0), stop=(tkp == TT - 1))

            pending = None    # issue PV one tk late so its exp-wait is
            for tk in range(TT):   # already satisfied at the queue head
                pool = sa_ps if tk % 2 == 0 else sb_ps
                ps = pool.tile([128, 2, 512], F32,
                               tag="sa" if tk % 2 == 0 else "sb")
                for h01 in range(2):
                    pb = h01 * D
                    nc.tensor.matmul(
                        ps[:, h01, :],
                        kTc[tk // 4][pb:pb + D, hp, bass.ts(tk % 4, 128)],
                        qTc[qc][pb:pb + D, hp, :],
                        start=True, stop=True)
                e_t = e_pool.tile([128, 2, 512], BF16, tag="e")
                if tk % EXP_DVE_EVERY == EXP_DVE_EVERY - 1:
                    nc.vector.tensor_scalar(e_t.bitcast(I16), ps,
                                            A16, B16, ALU.mult, ALU.add)
                else:
                    nc.scalar.activation(e_t, ps, AF.Exp, scale=SCALE)
                if pending is not None:
                    emit_pv(*pending)
                pending = (e_t, tk)
                yield
            emit_pv(*pending)
            yield
            # Evacuate po to SBUF now (frees the pv psum tiles for the next
            # c_iter's PV almost immediately); the rest of the normalization
            # is staged into norm_q, pumped one op per tk during the next
            # c_iter so the serial chain never blocks the DVE-exp stream.
            pocs = []
            for h01 in range(2):
                poc = nrm_pool.tile([128, 512], F32, tag="poc",
                                    name=f"poc{qc}_{hp}_{h01}")
                nc.vector.tensor_copy(poc[0:VS, :], po[h01][0:VS, :])
                pocs.append(poc)

            def stage(h01):
                poc = pocs[h01]
                den_s = nrm_pool.tile([1, 2, 512], F32, tag="dens")
                rec_b = nrm_pool.tile([64, 512], F32, tag="recb")
                yield lambda: nc.vector.tensor_mul(
                    den_s[:, 0, :], poc[D:D + 1, :], ones_r[D:D + 1, :])
                yield lambda: nc.vector.reciprocal_approx_fast(
                    den_s[:, 1, :], den_s[:, 0, :])
                yield lambda: nc.gpsimd.partition_broadcast(
                    rec_b, den_s[:, 1, :])
                yield lambda: nc.vector.tensor_mul(
                    oc[qc][64 * h01:64 * h01 + 64, hp, :],
                    poc[0:D, :], rec_b)

            norm_q.extend(list(stage(0)) + list(stage(1)))

        # ---- emission schedule (software pipeline) --------------------
        # Critical-path DMAs first (Q-proj inputs), everything prefetched up
        # front so proj matmuls never head-of-line block the tensor queue.
        nc.sync.dma_start(out=wq_s, in_=wq.rearrange("(kt p) c -> p kt c", p=128))
        xq_ts = {0: load_x(xqT[:, bass.ts(0, MC)], "xq0")}
        nc.sync.dma_start(out=wk_s, in_=wk.rearrange("(kt p) c -> p kt c", p=128))
        xk_ts = {0: load_x(xkT[:, bass.ts(0, MC)], "xk0")}
        nc.sync.dma_start(
            out=bq_s, in_=bq.rearrange("(mt p) -> p mt", p=128).unsqueeze(2))
        nc.sync.dma_start(
            out=bk_s, in_=bk.rearrange("(mt p) -> p mt", p=128).unsqueeze(2))
        nc.sync.dma_start(out=wv_s, in_=wv.rearrange("(kt p) c -> p kt c", p=128))
        nc.sync.dma_start(
            out=bv_bc,
            in_=bass.AP(tensor=bv.tensor, offset=bv.offset,
                        ap=[[0, 128]] + list(bv.ap)))
        xv_ts = {0: load_x(xvT[:, bass.ts(0, MC)], "xv0")}
        # touch Exp once so the ACT table loads during the DMA-bound head
        warm = consts.tile([128, 1], F32, tag="warm")
        nc.vector.memset(warm[0:1, :], 0.0)
        nc.scalar.activation(warm[0:1, :], warm[0:1, :], AF.Exp)
        ones_r = consts.tile([128, 512], F32, tag="onesr")
        nc.vector.memset(ones_r[D:D + 1, :], 1.0)
        # PE warm-up: dummy matmuls during the DMA head ramp the p-state
        # (PE reaches max clock after ~3us of continuous execution).
        wx = consts.tile([128, 512], BF16, tag="wx")
        nc.vector.memset(wx[:, 0:8], 0.0)
        psd = pp.tile([128, 512], F32, tag="pp", name="warmps")
        for i in range(16):
            nc.tensor.matmul(psd[0:8, :], wx[:, 0:8], wx,
                             start=(i == 0), stop=(i == 15))
        for mc in range(1, NMC):
            xq_ts[mc] = load_x(xqT[:, bass.ts(mc, MC)], f"xq{mc}")
            xk_ts[mc] = load_x(xkT[:, bass.ts(mc, MC)], f"xk{mc}")
            xv_ts[mc] = load_x(xvT[:, bass.ts(mc, MC)], f"xv{mc}")
        nc.sync.dma_start(out=wo_s, in_=wo.rearrange("(g p) c -> p g c", p=128))

        # chunk 0 proj fully, before attention can start
        for _ in emit_a(0, xq_ts[0], xk_ts[0]):
            pass
        for _ in emit_b(0, xv_ts[0]):
            pass

        # filler stream: proj chunks 1-3 (quanta), then out-proj d-units
        def filler_stream():
            for mc in range(1, NMC):
                yield from emit_a(mc, xq_ts[mc], xk_ts[mc])
                yield from emit_b(mc, xv_ts[mc])

        fillers = filler_stream()
        proj_left = 3 * 16           # filler quanta in proj phase
        d_queue = []                 # (qc, tl, cc) units, 2 MMs each

        def pump(n):
            nonlocal proj_left
            for _ in range(n):
                if norm_q:
                    norm_q.pop(0)()
                elif next(fillers, StopIteration) is not StopIteration:
                    proj_left -= 1
                elif d_queue:
                    emit_d_unit(*d_queue.pop(0))
                else:
                    break

        for qc in range(NQC):
            for hp in range(HG // 2):
                gen = c_iter(qc, hp)
                if hp == 0 and qc > 0:       # qc-1 fully staged by now
                    d_queue += [(qc - 1, tl, cc)
                                for tl in range(4) for cc in range(2)]
                for _ in range(TT):
                    next(gen)
                    # proj phase: 4 quanta/tk keeps chunk mc fully emitted
                    # before attention reaches tk=4*mc (in-order queues would
                    # otherwise head-of-line deadlock); then 1 op/tk.
                    pump(4 if proj_left > 0 else 1)
                for _ in gen:                # final PV + po evac + staging
                    pass
        pump(10**6)
        d_queue += [(NQC - 1, tl, cc) for tl in range(4) for cc in range(2)]
        for u in d_queue:
            emit_d_unit(*u)


_NC_CACHE = None


def _get_program():
    global _NC_CACHE
    if _NC_CACHE is None:
        _NC_CACHE = build_mha_program()
    return _NC_CACHE


def make_in_maps(query, key, value, Wq, bq, Wk, bk, Wv, bv, Wo):
    q = np.asarray(query, np.float32).reshape(B, T, C)
    k = np.asarray(key, np.float32).reshape(B, T, C)
    v = np.asarray(value, np.float32).reshape(B, T, C)
    xT = {n: [np.ascontiguousarray(a[b].T).astype(ml_dtypes.bfloat16)
              for b in range(B)]
          for n, a in (("q", q), ("k", k), ("v", v))}
    in_maps = []
    for c in range(N_CORES):
        b, g = divmod(c, GROUPS)
        sl = slice(g * CG, (g + 1) * CG)
        in_maps.append({
            "xqT": xT["q"][b], "xkT": xT["k"][b], "xvT": xT["v"][b],
            "wq": np.ascontiguousarray(np.asarray(Wq, np.float32)[:, sl]).astype(ml_dtypes.bfloat16),
            "wk": np.ascontiguousarray(np.asarray(Wk, np.float32)[:, sl]).astype(ml_dtypes.bfloat16),
            "wv": np.ascontiguousarray(np.asarray(Wv, np.float32)[:, sl]).astype(ml_dtypes.bfloat16),
            "bq": np.ascontiguousarray(np.asarray(bq, np.float32)[sl]),
            "bk": np.ascontiguousarray(np.asarray(bk, np.float32)[sl]),
            "bv": np.ascontiguousarray(np.asarray(bv, np.float32)[sl]),
            "wo": np.ascontiguousarray(np.asarray(Wo, np.float32)[sl, :]).astype(ml_dtypes.bfloat16),
        })
    return in_maps


def assemble_output(results, bo):
    y = np.zeros((B, T, C), np.float32)
    for c, res in enumerate(results):
        y[c // GROUPS] += res["yp"]
    y += np.asarray(bo, np.float32)
    return y


def kernel(query, key, value, Wq, bq, Wk, bk, Wv, bv, Wo, bo):
    nc = _get_program()
    in_maps = make_in_maps(query, key, value, Wq, bq, Wk, bk, Wv, bv, Wo)
    res = run_bass_kernel_spmd(nc, in_maps, list(range(N_CORES)))
    return assemble_output(res.results, bo)


# revision 35
# speedup vs baseline: 1.1509x; 1.0359x over previous
"""Multi-head attention (B=2, T=2048, C=1024, H=16) on 8 trn2 cores.

Sharding: core c -> batch b = c//4, head-group g = c%4 (4 heads, proj cols
[g*256, (g+1)*256)).  Host pre-transposes per-batch inputs to feature-major
[C, T] so every device matmul has its contraction dim on SBUF partitions.
Each core computes a partial output  O_g @ Wo[g-rows]  [2048, 1024]; the
host sums the 4 partials per batch and adds bo.

v3 (vs the original): the out-proj contracts K=128 by stacking head pairs
on partitions (normalize writes head 2h+1 to partitions 64-127), halving
out-proj tensor time; the denominator broadcast matmuls are replaced by
DVE reciprocal + GpSimd partition_broadcast; Q/K projection evacuation
moved to the ACT engine (Identity+bias) to keep DVE headroom.  All value
tensors stay bf16 -- fp8 was measured to cost ~10% output error (attention
averaging passes per-element relative noise straight through).
"""

import ml_dtypes
import numpy as np


import concourse.bass as bass
import concourse.tile as tile
from concourse import bacc, mybir
from concourse.bass_utils import run_bass_kernel_spmd

B, T, C, H, D = 2, 2048, 1024, 16, 64
N_CORES = 8
GROUPS = 4          # head-groups (cores per batch)
HG = H // GROUPS    # heads per core = 4
CG = HG * D         # proj cols per core = 256
KT = C // 128       # contraction k-tiles = 8
SCALE = D ** -0.5   # 1/8

F32 = mybir.dt.float32
BF16 = mybir.dt.bfloat16
I16 = mybir.dt.int16
AF = mybir.ActivationFunctionType
ALU = mybir.AluOpType

# Schraudolph exp for the DVE: bf16 bitpattern = round(A16*s + B16) approximates
# exp(s/8).  Every other exp tile runs here: the serialized ACT exp chain
# (2x1105ns per 2 tk through the sa/sb double-buffer) is the steady-state
# cadence governor, and alternating engines pipelines the two exps.  The
# ±3% sawtooth on half the weights adds ~1.2% sigma -> ~1.6e-2 total rel err.
LOG2E = 1.4426950408889634
A16 = 128 * 0.125 * LOG2E
B16 = 128 * (127 - 0.0579)
EXP_DVE_EVERY = 2


def build_mha_program():
    """Build the SPMD Bass program (identical on all 8 cores)."""
    nc = bacc.Bacc("TRN2", target_bir_lowering=False, debug=False,
                   num_devices=N_CORES)

    xqT = nc.dram_tensor("xqT", (C, T), BF16, kind="ExternalInput").ap()
    xkT = nc.dram_tensor("xkT", (C, T), BF16, kind="ExternalInput").ap()
    xvT = nc.dram_tensor("xvT", (C, T), BF16, kind="ExternalInput").ap()
    wq = nc.dram_tensor("wq", (C, CG), BF16, kind="ExternalInput").ap()
    wk = nc.dram_tensor("wk", (C, CG), BF16, kind="ExternalInput").ap()
    wv = nc.dram_tensor("wv", (C, CG), BF16, kind="ExternalInput").ap()
    bq = nc.dram_tensor("bq", (CG,), F32, kind="ExternalInput").ap()
    bk = nc.dram_tensor("bk", (CG,), F32, kind="ExternalInput").ap()
    bv = nc.dram_tensor("bv", (CG,), F32, kind="ExternalInput").ap()
    wo = nc.dram_tensor("wo", (CG, C), BF16, kind="ExternalInput").ap()
    yp = nc.dram_tensor("yp", (T, C), F32, kind="ExternalOutput").ap()

    with tile.TileContext(nc) as tc:
        _emit(tc, xqT, xkT, xvT, wq, wk, wv, bq, bk, bv, wo, yp)
    nc.compile()
    return nc


def _emit(tc, xqT, xkT, xvT, wq, wk, wv, bq, bk, bv, wo, yp):
    nc = tc.nc
    MT = CG // 128            # stationary tiles per projection = 2
    MC = 512                  # chunk width (tokens) everywhere
    NMC = T // MC             # 4 chunks
    TT = T // 128             # 16 t-tiles
    QC = 512                  # q-chunk width in attention
    NQC = T // QC             # 4 q-chunks
    VS = D + 1                # 65: V cols + ones col per head

    norm_q = []   # staged normalization closures (pumped 1 per tk)

    from contextlib import ExitStack
    with ExitStack() as ctx:
        consts = ctx.enter_context(tc.tile_pool(name="consts", bufs=1))
        xs_pool = ctx.enter_context(tc.tile_pool(name="xs", bufs=12))
        big = ctx.enter_context(tc.tile_pool(name="big", bufs=1))
        e_pool = ctx.enter_context(tc.tile_pool(name="e", bufs=6))
        ev_pool = ctx.enter_context(tc.tile_pool(name="ev", bufs=3))
        nrm_pool = ctx.enter_context(tc.tile_pool(name="nrm", bufs=4))
        pp = ctx.enter_context(tc.tile_pool(name="pp", bufs=2, space="PSUM"))
        pv_ps = ctx.enter_context(tc.tile_pool(name="pvps", bufs=2, space="PSUM"))
        sa_ps = ctx.enter_context(tc.tile_pool(name="saps", bufs=1, space="PSUM"))
        sb_ps = ctx.enter_context(tc.tile_pool(name="sbps", bufs=1, space="PSUM"))

        # Per-chunk persistent activations: fine-grained tiles so stages
        # pipeline at chunk granularity instead of a hard phase boundary.
        qTc = [big.tile([128, MT, MC], BF16, name=f"qTc{i}", tag=f"qTc{i}")
               for i in range(NMC)]
        kTc = [big.tile([128, MT, MC], BF16, name=f"kTc{i}", tag=f"kTc{i}")
               for i in range(NMC)]
        vc = [big.tile([128, MC // 128, HG * VS], BF16, name=f"vc{i}",
                       tag=f"vc{i}") for i in range(NMC)]
        # out-proj stationary: head pair g2 stacked on partitions
        # (head 2*g2 -> partitions 0-63, head 2*g2+1 -> partitions 64-127)
        oc = [big.tile([128, HG // 2, QC], BF16, name=f"oc{i}", tag=f"oc{i}")
              for i in range(NQC)]

        wq_s = consts.tile([128, KT, CG], BF16, tag="wq")
        wk_s = consts.tile([128, KT, CG], BF16, tag="wk")
        wv_s = consts.tile([128, KT, CG], BF16, tag="wv")
        wo_s = consts.tile([128, HG // 2, C], BF16, tag="wo")
        bq_s = consts.tile([128, MT, 1], F32, tag="bq")
        bk_s = consts.tile([128, MT, 1], F32, tag="bk")
        bv_bc = consts.tile([128, CG], F32, tag="bv")

        def load_x(src, name):
            x_t = xs_pool.tile([128, KT, MC], BF16, tag="xs", name=name)
            nc.sync.dma_start(out=x_t,
                              in_=src.rearrange("(kt p) m -> p kt m", p=128))
            return x_t

        def emit_a(mc, xq_t, xk_t):
            """Q/K projection for chunk mc; yields every 4 matmuls."""
            for x_t, w_s, b_s, dstl in ((xq_t, wq_s, bq_s, qTc),
                                        (xk_t, wk_s, bk_s, kTc)):
                for mt in range(MT):
                    ps = pp.tile([128, 512], F32, tag="pp")
                    for kt in range(KT):
                        nc.tensor.matmul(
                            ps[:, :MC],
                            w_s[:, kt, bass.ts(mt, 128)],
                            x_t[:, kt, :],
                            start=(kt == 0), stop=(kt == KT - 1))
                        if kt == 3:
                            yield
                    nc.vector.tensor_scalar_add(
                        dstl[mc][:, mt, :], ps[:, :MC], b_s[:, mt, :])
                    yield

        def emit_b(mc, xv_t):
            """V projection for chunk mc; yields every 8 matmuls (2 subs)."""
            v4 = vc[mc].rearrange("p t (h c) -> p t h c", h=HG)
            nc.vector.memset(v4[:, :, :, D:VS], 1.0)
            for sub in range(MC // 128):
                ps = pp.tile([128, 512], F32, tag="pp")
                for kt in range(KT):
                    nc.tensor.matmul(
                        ps[:, :CG],
                        xv_t[:, kt, bass.ts(sub, 128)],
                        wv_s[:, kt, :],
                        start=(kt == 0), stop=(kt == KT - 1))
                    if kt == 3:
                        yield
                nc.vector.tensor_add(
                    v4[:, sub, :, 0:D],
                    ps[:, :CG].rearrange("p (h c) -> p h c", h=HG),
                    bv_bc.rearrange("p (h c) -> p h c", h=HG))
                yield

        def emit_d_unit(qc, tl, cc):
            trows_out = bass.ts(qc * (QC // 128) + tl, 128)
            ps = pp.tile([128, 512], F32, tag="pp")
            for g2 in range(HG // 2):
                nc.tensor.matmul(
                    ps,
                    oc[qc][:, g2, bass.ts(tl, 128)],
                    wo_s[:, g2, bass.ts(cc, 512)],
                    start=(g2 == 0), stop=(g2 == HG // 2 - 1))
            ev = ev_pool.tile([128, 512], F32, tag="ev")
            nc.scalar.copy(ev, ps)
            nc.sync.dma_start(out=yp[trows_out, bass.ts(cc, 512)], in_=ev)

        def c_iter(qc, hp):
            """Attention for head pair hp on q-chunk qc.  Yields after every
            tk so filler matmuls can interleave at fine granularity."""
            po = [pv_ps.tile([128, 512], F32, tag="pv",
                             name=f"po{qc}_{hp}_{i}") for i in range(2)]
            def emit_pv(e_prev, tkp):
                for h01 in range(2):
                    nc.tensor.matmul(
                        po[h01][0:VS, :],
                        vc[tkp // 4][:, tkp % 4,
                                     bass.ds((2 * hp + h01) * VS, VS)],
                        e_prev[:, h01, :],
                        start=(tkp == 0), stop=(tkp == TT - 1))

            pending = []      # issue PV two tks late so the exp latency
            for tk in range(TT):   # never gates the tensor chain
                pool = sa_ps if tk % 2 == 0 else sb_ps
                ps = pool.tile([128, 2, 512], F32,
                               tag="sa" if tk % 2 == 0 else "sb")
                for h01 in range(2):
                    pb = h01 * D
                    nc.tensor.matmul(
                        ps[:, h01, :],
                        kTc[tk // 4][pb:pb + D, hp, bass.ts(tk % 4, 128)],
                        qTc[qc][pb:pb + D, hp, :],
                        start=True, stop=True)
                e_t = e_pool.tile([128, 2, 512], BF16, tag="e")
                if tk % EXP_DVE_EVERY == EXP_DVE_EVERY - 1:
                    nc.vector.tensor_scalar(e_t.bitcast(I16), ps,
                                            A16, B16, ALU.mult, ALU.add)
                else:
                    nc.scalar.activation(e_t, ps, AF.Exp, scale=SCALE)
                pending.append((e_t, tk))
                if len(pending) > 2:
                    emit_pv(*pending.pop(0))
                yield
            for pe in pending:
                emit_pv(*pe)
            yield
            # Evacuate po to SBUF now (frees the pv psum tiles for the next
            # c_iter's PV almost immediately); the rest of the normalization
            # is staged into norm_q, pumped one op per tk during the next
            # c_iter so the serial chain never blocks the DVE-exp stream.
            pocs = []
            for h01 in range(2):
                poc = nrm_pool.tile([128, 512], F32, tag="poc",
                                    name=f"poc{qc}_{hp}_{h01}")
                nc.vector.tensor_copy(poc[0:VS, :], po[h01][0:VS, :])
                pocs.append(poc)

            def stage(h01):
                poc = pocs[h01]
                den_s = nrm_pool.tile([1, 2, 512], F32, tag="dens")
                rec_b = nrm_pool.tile([64, 512], F32, tag="recb")
                yield lambda: nc.vector.tensor_mul(
                    den_s[:, 0, :], poc[D:D + 1, :], ones_r[D:D + 1, :])
                yield lambda: nc.vector.reciprocal_approx_fast(
                    den_s[:, 1, :], den_s[:, 0, :])
                yield lambda: nc.gpsimd.partition_broadcast(
                    rec_b, den_s[:, 1, :])
                yield lambda: nc.vector.tensor_mul(
                    oc[qc][64 * h01:64 * h01 + 64, hp, :],
                    poc[0:D, :], rec_b)

            norm_q.extend(list(stage(0)) + list(stage(1)))

        # ---- emission schedule (software pipeline) --------------------
        # Critical-path DMAs first (Q-proj inputs), everything prefetched up
        # front so proj matmuls never head-of-line block the tensor queue.
        nc.sync.dma_start(out=wq_s, in_=wq.rearrange("(kt p) c -> p kt c", p=128))
        xq_ts = {0: load_x(xqT[:, bass.ts(0, MC)], "xq0")}
        nc.sync.dma_start(out=wk_s, in_=wk.rearrange("(kt p) c -> p kt c", p=128))
        xk_ts = {0: load_x(xkT[:, bass.ts(0, MC)], "xk0")}
        nc.sync.dma_start(
            out=bq_s, in_=bq.rearrange("(mt p) -> p mt", p=128).unsqueeze(2))
        nc.sync.dma_start(
            out=bk_s, in_=bk.rearrange("(mt p) -> p mt", p=128).unsqueeze(2))
        nc.sync.dma_start(out=wv_s, in_=wv.rearrange("(kt p) c -> p kt c", p=128))
        nc.sync.dma_start(
            out=bv_bc,
            in_=bass.AP(tensor=bv.tensor, offset=bv.offset,
                        ap=[[0, 128]] + list(bv.ap)))
        xv_ts = {0: load_x(xvT[:, bass.ts(0, MC)], "xv0")}
        # touch Exp once so the ACT table loads during the DMA-bound head
        warm = consts.tile([128, 1], F32, tag="warm")
        nc.vector.memset(warm[0:1, :], 0.0)
        nc.scalar.activation(warm[0:1, :], warm[0:1, :], AF.Exp)
        ones_r = consts.tile([128, 512], F32, tag="onesr")
        nc.vector.memset(ones_r[D:D + 1, :], 1.0)
        # PE warm-up: dummy matmuls during the DMA head ramp the p-state
        # (PE reaches max clock after ~3us of continuous execution).
        wx = consts.tile([128, 512], BF16, tag="wx")
        nc.vector.memset(wx[:, 0:8], 0.0)
        psd = pp.tile([128, 512], F32, tag="pp", name="warmps")
        for i in range(16):
            nc.tensor.matmul(psd[0:8, :], wx[:, 0:8], wx,
                             start=(i == 0), stop=(i == 15))
        for mc in range(1, NMC):
            xq_ts[mc] = load_x(xqT[:, bass.ts(mc, MC)], f"xq{mc}")
            xk_ts[mc] = load_x(xkT[:, bass.ts(mc, MC)], f"xk{mc}")
            xv_ts[mc] = load_x(xvT[:, bass.ts(mc, MC)], f"xv{mc}")
        nc.sync.dma_start(out=wo_s, in_=wo.rearrange("(g p) c -> p g c", p=128))

        # chunk 0 proj fully, before attention can start
        for _ in emit_a(0, xq_ts[0], xk_ts[0]):
            pass
        for _ in emit_b(0, xv_ts[0]):
            pass

        # filler stream: proj chunks 1-3 (quanta), then out-proj d-units
        def filler_stream():
            for mc in range(1, NMC):
                yield from emit_a(mc, xq_ts[mc], xk_ts[mc])
                yield from emit_b(mc, xv_ts[mc])

        fillers = filler_stream()
        proj_left = 3 * 16           # filler quanta in proj phase
        d_queue = []                 # (qc, tl, cc) units, 2 MMs each

        def pump(n):
            nonlocal proj_left
            for _ in range(n):
                if norm_q:
                    norm_q.pop(0)()
                elif next(fillers, StopIteration) is not StopIteration:
                    proj_left -= 1
                elif d_queue:
                    emit_d_unit(*d_queue.pop(0))
                else:
                    break

        for qc in range(NQC):
            for hp in range(HG // 2):
                gen = c_iter(qc, hp)
                if hp == 0 and qc > 0:       # qc-1 fully staged by now
                    d_queue += [(qc - 1, tl, cc)
                                for tl in range(4) for cc in range(2)]
                for _ in range(TT):
                    next(gen)
                    # proj phase: 4 quanta/tk keeps chunk mc fully emitted
                    # before attention reaches tk=4*mc (in-order queues would
                    # otherwise head-of-line deadlock); then 1 op/tk.
                    pump(4 if proj_left > 0 else 1)
                for _ in gen:                # final PV + po evac + staging
                    pass
        pump(10**6)
        d_queue += [(NQC - 1, tl, cc) for tl in range(4) for cc in range(2)]
        for u in d_queue:
            emit_d_unit(*u)


_NC_CACHE = None


def _get_program():
    global _NC_CACHE
    if _NC_CACHE is None:
        _NC_CACHE = build_mha_program()
    return _NC_CACHE


def make_in_maps(query, key, value, Wq, bq, Wk, bk, Wv, bv, Wo):
    q = np.asarray(query, np.float32).reshape(B, T, C)
    k = np.asarray(key, np.float32).reshape(B, T, C)
    v = np.asarray(value, np.float32).reshape(B, T, C)
    xT = {n: [np.ascontiguousarray(a[b].T).astype(ml_dtypes.bfloat16)
              for b in range(B)]
          for n, a in (("q", q), ("k", k), ("v", v))}
    in_maps = []
    for c in range(N_CORES):
        b, g = divmod(c, GROUPS)
        sl = slice(g * CG, (g + 1) * CG)
        in_maps.append({
            "xqT": xT["q"][b], "xkT": xT["k"][b], "xvT": xT["v"][b],
            "wq": np.ascontiguousarray(np.asarray(Wq, np.float32)[:, sl]).astype(ml_dtypes.bfloat16),
            "wk": np.ascontiguousarray(np.asarray(Wk, np.float32)[:, sl]).astype(ml_dtypes.bfloat16),
            "wv": np.ascontiguousarray(np.asarray(Wv, np.float32)[:, sl]).astype(ml_dtypes.bfloat16),
            "bq": np.ascontiguousarray(np.asarray(bq, np.float32)[sl]),
            "bk": np.ascontiguousarray(np.asarray(bk, np.float32)[sl]),
            "bv": np.ascontiguousarray(np.asarray(bv, np.float32)[sl]),
            "wo": np.ascontiguousarray(np.asarray(Wo, np.float32)[sl, :]).astype(ml_dtypes.bfloat16),
        })
    return in_maps


def assemble_output(results, bo):
    y = np.zeros((B, T, C), np.float32)
    for c, res in enumerate(results):
        y[c // GROUPS] += res["yp"]
    y += np.asarray(bo, np.float32)
    return y


def kernel(query, key, value, Wq, bq, Wk, bk, Wv, bv, Wo, bo):
    nc = _get_program()
    in_maps = make_in_maps(query, key, value, Wq, bq, Wk, bk, Wv, bv, Wo)
    res = run_bass_kernel_spmd(nc, in_maps, list(range(N_CORES)))
    return assemble_output(res.results, bo)


# revision 36
# speedup vs baseline: 1.1837x; 1.0285x over previous
"""Multi-head attention (B=2, T=2048, C=1024, H=16) on 8 trn2 cores.

Sharding: core c -> batch b = c//4, head-group g = c%4 (4 heads, proj cols
[g*256, (g+1)*256)).  Host pre-transposes per-batch inputs to feature-major
[C, T] so every device matmul has its contraction dim on SBUF partitions.
Each core computes a partial output  O_g @ Wo[g-rows]  [2048, 1024]; the
host sums the 4 partials per batch and adds bo.

v3 (vs the original): the out-proj contracts K=128 by stacking head pairs
on partitions (normalize writes head 2h+1 to partitions 64-127), halving
out-proj tensor time; the denominator broadcast matmuls are replaced by
DVE reciprocal + GpSimd partition_broadcast; Q/K projection evacuation
moved to the ACT engine (Identity+bias) to keep DVE headroom.  All value
tensors stay bf16 -- fp8 was measured to cost ~10% output error (attention
averaging passes per-element relative noise straight through).
"""

import ml_dtypes
import numpy as np


import concourse.bass as bass
import concourse.tile as tile
from concourse import bacc, mybir
from concourse.bass_utils import run_bass_kernel_spmd

B, T, C, H, D = 2, 2048, 1024, 16, 64
N_CORES = 8
GROUPS = 4          # head-groups (cores per batch)
HG = H // GROUPS    # heads per core = 4
CG = HG * D         # proj cols per core = 256
KT = C // 128       # contraction k-tiles = 8
SCALE = D ** -0.5   # 1/8

F32 = mybir.dt.float32
BF16 = mybir.dt.bfloat16
I16 = mybir.dt.int16
AF = mybir.ActivationFunctionType
ALU = mybir.AluOpType

# Schraudolph exp for the DVE: bf16 bitpattern = round(A16*s + B16) approximates
# exp(s/8).  Every other exp tile runs here: the serialized ACT exp chain
# (2x1105ns per 2 tk through the sa/sb double-buffer) is the steady-state
# cadence governor, and alternating engines pipelines the two exps.  The
# ±3% sawtooth on half the weights adds ~1.2% sigma -> ~1.6e-2 total rel err.
LOG2E = 1.4426950408889634
A16 = 128 * 0.125 * LOG2E
B16 = 128 * (127 - 0.0579)
EXP_DVE_EVERY = 10**9


def build_mha_program():
    """Build the SPMD Bass program (identical on all 8 cores)."""
    nc = bacc.Bacc("TRN2", target_bir_lowering=False, debug=False,
                   num_devices=N_CORES)

    xqT = nc.dram_tensor("xqT", (C, T), BF16, kind="ExternalInput").ap()
    xkT = nc.dram_tensor("xkT", (C, T), BF16, kind="ExternalInput").ap()
    xvT = nc.dram_tensor("xvT", (C, T), BF16, kind="ExternalInput").ap()
    wq = nc.dram_tensor("wq", (C, CG), BF16, kind="ExternalInput").ap()
    wk = nc.dram_tensor("wk", (C, CG), BF16, kind="ExternalInput").ap()
    wv = nc.dram_tensor("wv", (C, CG), BF16, kind="ExternalInput").ap()
    bq = nc.dram_tensor("bq", (CG,), F32, kind="ExternalInput").ap()
    bk = nc.dram_tensor("bk", (CG,), F32, kind="ExternalInput").ap()
    bv = nc.dram_tensor("bv", (CG,), F32, kind="ExternalInput").ap()
    wo = nc.dram_tensor("wo", (CG, C), BF16, kind="ExternalInput").ap()
    yp = nc.dram_tensor("yp", (T, C), F32, kind="ExternalOutput").ap()

    with tile.TileContext(nc) as tc:
        _emit(tc, xqT, xkT, xvT, wq, wk, wv, bq, bk, bv, wo, yp)
    nc.compile()
    return nc


def _emit(tc, xqT, xkT, xvT, wq, wk, wv, bq, bk, bv, wo, yp):
    nc = tc.nc
    MT = CG // 128            # stationary tiles per projection = 2
    MC = 512                  # chunk width (tokens) everywhere
    NMC = T // MC             # 4 chunks
    TT = T // 128             # 16 t-tiles
    QC = 512                  # q-chunk width in attention
    NQC = T // QC             # 4 q-chunks
    VS = D + 1                # 65: V cols + ones col per head

    norm_q = []   # staged normalization closures (pumped 1 per tk)

    from contextlib import ExitStack
    with ExitStack() as ctx:
        consts = ctx.enter_context(tc.tile_pool(name="consts", bufs=1))
        xs_pool = ctx.enter_context(tc.tile_pool(name="xs", bufs=12))
        big = ctx.enter_context(tc.tile_pool(name="big", bufs=1))
        e_pool = ctx.enter_context(tc.tile_pool(name="e", bufs=6))
        ev_pool = ctx.enter_context(tc.tile_pool(name="ev", bufs=3))
        nrm_pool = ctx.enter_context(tc.tile_pool(name="nrm", bufs=4))
        pp = ctx.enter_context(tc.tile_pool(name="pp", bufs=2, space="PSUM"))
        pv_ps = ctx.enter_context(tc.tile_pool(name="pvps", bufs=2, space="PSUM"))
        sa_ps = ctx.enter_context(tc.tile_pool(name="saps", bufs=1, space="PSUM"))
        sb_ps = ctx.enter_context(tc.tile_pool(name="sbps", bufs=1, space="PSUM"))

        # Per-chunk persistent activations: fine-grained tiles so stages
        # pipeline at chunk granularity instead of a hard phase boundary.
        qTc = [big.tile([128, MT, MC], BF16, name=f"qTc{i}", tag=f"qTc{i}")
               for i in range(NMC)]
        kTc = [big.tile([128, MT, MC], BF16, name=f"kTc{i}", tag=f"kTc{i}")
               for i in range(NMC)]
        vc = [big.tile([128, MC // 128, HG * VS], BF16, name=f"vc{i}",
                       tag=f"vc{i}") for i in range(NMC)]
        # out-proj stationary: head pair g2 stacked on partitions
        # (head 2*g2 -> partitions 0-63, head 2*g2+1 -> partitions 64-127)
        oc = [big.tile([128, HG // 2, QC], BF16, name=f"oc{i}", tag=f"oc{i}")
              for i in range(NQC)]

        wq_s = consts.tile([128, KT, CG], BF16, tag="wq")
        wk_s = consts.tile([128, KT, CG], BF16, tag="wk")
        wv_s = consts.tile([128, KT, CG], BF16, tag="wv")
        wo_s = consts.tile([128, HG // 2, C], BF16, tag="wo")
        bq_s = consts.tile([128, MT, 1], F32, tag="bq")
        bk_s = consts.tile([128, MT, 1], F32, tag="bk")
        bv_bc = consts.tile([128, CG], F32, tag="bv")

        def load_x(src, name):
            x_t = xs_pool.tile([128, KT, MC], BF16, tag="xs", name=name)
            nc.sync.dma_start(out=x_t,
                              in_=src.rearrange("(kt p) m -> p kt m", p=128))
            return x_t

        def emit_a(mc, xq_t, xk_t):
            """Q/K projection for chunk mc; yields every 4 matmuls."""
            for x_t, w_s, b_s, dstl in ((xq_t, wq_s, bq_s, qTc),
                                        (xk_t, wk_s, bk_s, kTc)):
                for mt in range(MT):
                    ps = pp.tile([128, 512], F32, tag="pp")
                    for kt in range(KT):
                        nc.tensor.matmul(
                            ps[:, :MC],
                            w_s[:, kt, bass.ts(mt, 128)],
                            x_t[:, kt, :],
                            start=(kt == 0), stop=(kt == KT - 1))
                        if kt == 3:
                            yield
                    nc.vector.tensor_scalar_add(
                        dstl[mc][:, mt, :], ps[:, :MC], b_s[:, mt, :])
                    yield

        def emit_b(mc, xv_t):
            """V projection for chunk mc; yields every 8 matmuls (2 subs)."""
            v4 = vc[mc].rearrange("p t (h c) -> p t h c", h=HG)
            nc.vector.memset(v4[:, :, :, D:VS], 1.0)
            for sub in range(MC // 128):
                ps = pp.tile([128, 512], F32, tag="pp")
                for kt in range(KT):
                    nc.tensor.matmul(
                        ps[:, :CG],
                        xv_t[:, kt, bass.ts(sub, 128)],
                        wv_s[:, kt, :],
                        start=(kt == 0), stop=(kt == KT - 1))
                    if kt == 3:
                        yield
                nc.vector.tensor_add(
                    v4[:, sub, :, 0:D],
                    ps[:, :CG].rearrange("p (h c) -> p h c", h=HG),
                    bv_bc.rearrange("p (h c) -> p h c", h=HG))
                yield

        def emit_d_unit(qc, tl, cc):
            trows_out = bass.ts(qc * (QC // 128) + tl, 128)
            ps = pp.tile([128, 512], F32, tag="pp")
            for g2 in range(HG // 2):
                nc.tensor.matmul(
                    ps,
                    oc[qc][:, g2, bass.ts(tl, 128)],
                    wo_s[:, g2, bass.ts(cc, 512)],
                    start=(g2 == 0), stop=(g2 == HG // 2 - 1))
            ev = ev_pool.tile([128, 512], F32, tag="ev")
            nc.vector.tensor_copy(ev, ps)
            nc.sync.dma_start(out=yp[trows_out, bass.ts(cc, 512)], in_=ev)

        def c_iter(qc, hp):
            """Attention for head pair hp on q-chunk qc.  Yields after every
            tk so filler matmuls can interleave at fine granularity."""
            po = [pv_ps.tile([128, 512], F32, tag="pv",
                             name=f"po{qc}_{hp}_{i}") for i in range(2)]
            def emit_pv(e_prev, tkp):
                for h01 in range(2):
                    nc.tensor.matmul(
                        po[h01][0:VS, :],
                        vc[tkp // 4][:, tkp % 4,
                                     bass.ds((2 * hp + h01) * VS, VS)],
                        e_prev[:, h01, :],
                        start=(tkp == 0), stop=(tkp == TT - 1))

            pending = []      # issue PV two tks late so the exp latency
            for tk in range(TT):   # never gates the tensor chain
                pool = sa_ps if tk % 2 == 0 else sb_ps
                ps = pool.tile([128, 2, 512], F32,
                               tag="sa" if tk % 2 == 0 else "sb")
                for h01 in range(2):
                    pb = h01 * D
                    nc.tensor.matmul(
                        ps[:, h01, :],
                        kTc[tk // 4][pb:pb + D, hp, bass.ts(tk % 4, 128)],
                        qTc[qc][pb:pb + D, hp, :],
                        start=True, stop=True)
                e_t = e_pool.tile([128, 2, 512], BF16, tag="e")
                if tk % EXP_DVE_EVERY == EXP_DVE_EVERY - 1:
                    nc.vector.tensor_scalar(e_t.bitcast(I16), ps,
                                            A16, B16, ALU.mult, ALU.add)
                else:
                    nc.scalar.activation(e_t, ps, AF.Exp, scale=SCALE)
                pending.append((e_t, tk))
                if len(pending) > 2:
                    emit_pv(*pending.pop(0))
                yield
            for pe in pending:
                emit_pv(*pe)
            yield
            # Evacuate po to SBUF now (frees the pv psum tiles for the next
            # c_iter's PV almost immediately); the rest of the normalization
            # is staged into norm_q, pumped one op per tk during the next
            # c_iter so the serial chain never blocks the DVE-exp stream.
            pocs = []
            for h01 in range(2):
                poc = nrm_pool.tile([128, 512], F32, tag="poc",
                                    name=f"poc{qc}_{hp}_{h01}")
                nc.vector.tensor_copy(poc[0:VS, :], po[h01][0:VS, :])
                pocs.append(poc)

            def stage(h01):
                poc = pocs[h01]
                den_s = nrm_pool.tile([1, 2, 512], F32, tag="dens")
                rec_b = nrm_pool.tile([64, 512], F32, tag="recb")
                yield lambda: nc.vector.tensor_mul(
                    den_s[:, 0, :], poc[D:D + 1, :], ones_r[D:D + 1, :])
                yield lambda: nc.vector.reciprocal_approx_fast(
                    den_s[:, 1, :], den_s[:, 0, :])
                yield lambda: nc.gpsimd.partition_broadcast(
                    rec_b, den_s[:, 1, :])
                yield lambda: nc.vector.tensor_mul(
                    oc[qc][64 * h01:64 * h01 + 64, hp, :],
                    poc[0:D, :], rec_b)

            norm_q.extend(list(stage(0)) + list(stage(1)))

        # ---- emission schedule (software pipeline) --------------------
        # Critical-path DMAs first (Q-proj inputs), everything prefetched up
        # front so proj matmuls never head-of-line block the tensor queue.
        nc.sync.dma_start(out=wq_s, in_=wq.rearrange("(kt p) c -> p kt c", p=128))
        xq_ts = {0: load_x(xqT[:, bass.ts(0, MC)], "xq0")}
        nc.sync.dma_start(out=wk_s, in_=wk.rearrange("(kt p) c -> p kt c", p=128))
        xk_ts = {0: load_x(xkT[:, bass.ts(0, MC)], "xk0")}
        nc.sync.dma_start(
            out=bq_s, in_=bq.rearrange("(mt p) -> p mt", p=128).unsqueeze(2))
        nc.sync.dma_start(
            out=bk_s, in_=bk.rearrange("(mt p) -> p mt", p=128).unsqueeze(2))
        nc.sync.dma_start(out=wv_s, in_=wv.rearrange("(kt p) c -> p kt c", p=128))
        nc.sync.dma_start(
            out=bv_bc,
            in_=bass.AP(tensor=bv.tensor, offset=bv.offset,
                        ap=[[0, 128]] + list(bv.ap)))
        xv_ts = {0: load_x(xvT[:, bass.ts(0, MC)], "xv0")}
        # touch Exp once so the ACT table loads during the DMA-bound head
        warm = consts.tile([128, 1], F32, tag="warm")
        nc.vector.memset(warm[0:1, :], 0.0)
        nc.scalar.activation(warm[0:1, :], warm[0:1, :], AF.Exp)
        ones_r = consts.tile([128, 512], F32, tag="onesr")
        nc.vector.memset(ones_r[D:D + 1, :], 1.0)
        # PE warm-up: dummy matmuls during the DMA head ramp the p-state
        # (PE reaches max clock after ~3us of continuous execution).
        wx = consts.tile([128, 512], BF16, tag="wx")
        nc.vector.memset(wx[:, 0:8], 0.0)
        psd = pp.tile([128, 512], F32, tag="pp", name="warmps")
        for i in range(16):
            nc.tensor.matmul(psd[0:8, :], wx[:, 0:8], wx,
                             start=(i == 0), stop=(i == 15))
        for mc in range(1, NMC):
            xq_ts[mc] = load_x(xqT[:, bass.ts(mc, MC)], f"xq{mc}")
            xk_ts[mc] = load_x(xkT[:, bass.ts(mc, MC)], f"xk{mc}")
            xv_ts[mc] = load_x(xvT[:, bass.ts(mc, MC)], f"xv{mc}")
        nc.sync.dma_start(out=wo_s, in_=wo.rearrange("(g p) c -> p g c", p=128))

        # chunk 0 proj fully, before attention can start
        for _ in emit_a(0, xq_ts[0], xk_ts[0]):
            pass
        for _ in emit_b(0, xv_ts[0]):
            pass

        # filler stream: proj chunks 1-3 (quanta), then out-proj d-units
        def filler_stream():
            for mc in range(1, NMC):
                yield from emit_a(mc, xq_ts[mc], xk_ts[mc])
                yield from emit_b(mc, xv_ts[mc])

        fillers = filler_stream()
        proj_left = 3 * 16           # filler quanta in proj phase
        d_queue = []                 # (qc, tl, cc) units, 2 MMs each

        def pump(n):
            nonlocal proj_left
            for _ in range(n):
                if norm_q:
                    norm_q.pop(0)()
                elif next(fillers, StopIteration) is not StopIteration:
                    proj_left -= 1
                elif d_queue:
                    emit_d_unit(*d_queue.pop(0))
                else:
                    break

        for qc in range(NQC):
            for hp in range(HG // 2):
                gen = c_iter(qc, hp)
                if hp == 0 and qc > 0:       # qc-1 fully staged by now
                    d_queue += [(qc - 1, tl, cc)
                                for tl in range(4) for cc in range(2)]
                for _ in range(TT):
                    next(gen)
                    # proj phase: 4 quanta/tk keeps chunk mc fully emitted
                    # before attention reaches tk=4*mc (in-order queues would
                    # otherwise head-of-line deadlock); then 1 op/tk.
                    pump(4 if proj_left > 0 else 1)
                for _ in gen:                # final PV + po evac + staging
                    pass
        pump(10**6)
        d_queue += [(NQC - 1, tl, cc) for tl in range(4) for cc in range(2)]
        for u in d_queue:
            emit_d_unit(*u)


_NC_CACHE = None


def _get_program():
    global _NC_CACHE
    if _NC_CACHE is None:
        _NC_CACHE = build_mha_program()
    return _NC_CACHE


def make_in_maps(query, key, value, Wq, bq, Wk, bk, Wv, bv, Wo):
    q = np.asarray(query, np.float32).reshape(B, T, C)
    k = np.asarray(key, np.float32).reshape(B, T, C)
    v = np.asarray(value, np.float32).reshape(B, T, C)
    xT = {n: [np.ascontiguousarray(a[b].T).astype(ml_dtypes.bfloat16)
              for b in range(B)]
          for n, a in (("q", q), ("k", k), ("v", v))}
    in_maps = []
    for c in range(N_CORES):
        b, g = divmod(c, GROUPS)
        sl = slice(g * CG, (g + 1) * CG)
        in_maps.append({
            "xqT": xT["q"][b], "xkT": xT["k"][b], "xvT": xT["v"][b],
            "wq": np.ascontiguousarray(np.asarray(Wq, np.float32)[:, sl]).astype(ml_dtypes.bfloat16),
            "wk": np.ascontiguousarray(np.asarray(Wk, np.float32)[:, sl]).astype(ml_dtypes.bfloat16),
            "wv": np.ascontiguousarray(np.asarray(Wv, np.float32)[:, sl]).astype(ml_dtypes.bfloat16),
            "bq": np.ascontiguousarray(np.asarray(bq, np.float32)[sl]),
            "bk": np.ascontiguousarray(np.asarray(bk, np.float32)[sl]),
            "bv": np.ascontiguousarray(np.asarray(bv, np.float32)[sl]),
            "wo": np.ascontiguousarray(np.asarray(Wo, np.float32)[sl, :]).astype(ml_dtypes.bfloat16),
        })
    return in_maps


def assemble_output(results, bo):
    y = np.zeros((B, T, C), np.float32)
    for c, res in enumerate(results):
        y[c // GROUPS] += res["yp"]
    y += np.asarray(bo, np.float32)
    return y


def kernel(query, key, value, Wq, bq, Wk, bk, Wv, bv, Wo, bo):
    nc = _get_program()
    in_maps = make_in_maps(query, key, value, Wq, bq, Wk, bk, Wv, bv, Wo)
    res = run_bass_kernel_spmd(nc, in_maps, list(range(N_CORES)))
    return assemble_output(res.results, bo)


# revision 40
# speedup vs baseline: 1.1853x; 1.0014x over previous
"""Multi-head attention (B=2, T=2048, C=1024, H=16) on 8 trn2 cores.

Sharding: core c -> batch b = c//4, head-group g = c%4 (4 heads, proj cols
[g*256, (g+1)*256)).  Host pre-transposes per-batch inputs to feature-major
[C, T] so every device matmul has its contraction dim on SBUF partitions.
Each core computes a partial output  O_g @ Wo[g-rows]  [2048, 1024]; the
host sums the 4 partials per batch and adds bo.

v3 (vs the original): the out-proj contracts K=128 by stacking head pairs
on partitions (normalize writes head 2h+1 to partitions 64-127), halving
out-proj tensor time; the denominator broadcast matmuls are replaced by
DVE reciprocal + GpSimd partition_broadcast; Q/K projection evacuation
moved to the ACT engine (Identity+bias) to keep DVE headroom.  All value
tensors stay bf16 -- fp8 was measured to cost ~10% output error (attention
averaging passes per-element relative noise straight through).
"""

import ml_dtypes
import numpy as np


import concourse.bass as bass
import concourse.tile as tile
from concourse import bacc, mybir
from concourse.bass_utils import run_bass_kernel_spmd

B, T, C, H, D = 2, 2048, 1024, 16, 64
N_CORES = 8
GROUPS = 4          # head-groups (cores per batch)
HG = H // GROUPS    # heads per core = 4
CG = HG * D         # proj cols per core = 256
KT = C // 128       # contraction k-tiles = 8
SCALE = D ** -0.5   # 1/8

F32 = mybir.dt.float32
BF16 = mybir.dt.bfloat16
I16 = mybir.dt.int16
AF = mybir.ActivationFunctionType
ALU = mybir.AluOpType

# Schraudolph exp for the DVE: bf16 bitpattern = round(A16*s + B16) approximates
# exp(s/8).  Every other exp tile runs here: the serialized ACT exp chain
# (2x1105ns per 2 tk through the sa/sb double-buffer) is the steady-state
# cadence governor, and alternating engines pipelines the two exps.  The
# ±3% sawtooth on half the weights adds ~1.2% sigma -> ~1.6e-2 total rel err.
LOG2E = 1.4426950408889634
A16 = 128 * 0.125 * LOG2E
B16 = 128 * (127 - 0.0579)
EXP_DVE_EVERY = 10**9


def build_mha_program():
    """Build the SPMD Bass program (identical on all 8 cores)."""
    nc = bacc.Bacc("TRN2", target_bir_lowering=False, debug=False,
                   num_devices=N_CORES)

    xqT = nc.dram_tensor("xqT", (C, T), BF16, kind="ExternalInput").ap()
    xkT = nc.dram_tensor("xkT", (C, T), BF16, kind="ExternalInput").ap()
    xvT = nc.dram_tensor("xvT", (C, T), BF16, kind="ExternalInput").ap()
    wq = nc.dram_tensor("wq", (C, CG), BF16, kind="ExternalInput").ap()
    wk = nc.dram_tensor("wk", (C, CG), BF16, kind="ExternalInput").ap()
    wv = nc.dram_tensor("wv", (C, CG), BF16, kind="ExternalInput").ap()
    bq = nc.dram_tensor("bq", (CG,), F32, kind="ExternalInput").ap()
    bk = nc.dram_tensor("bk", (CG,), F32, kind="ExternalInput").ap()
    bv = nc.dram_tensor("bv", (CG,), F32, kind="ExternalInput").ap()
    wo = nc.dram_tensor("wo", (CG, C), BF16, kind="ExternalInput").ap()
    yp = nc.dram_tensor("yp", (T, C), F32, kind="ExternalOutput").ap()

    with tile.TileContext(nc) as tc:
        _emit(tc, xqT, xkT, xvT, wq, wk, wv, bq, bk, bv, wo, yp)
    nc.compile()
    return nc


def _emit(tc, xqT, xkT, xvT, wq, wk, wv, bq, bk, bv, wo, yp):
    nc = tc.nc
    MT = CG // 128            # stationary tiles per projection = 2
    MC = 512                  # chunk width (tokens) everywhere
    NMC = T // MC             # 4 chunks
    TT = T // 128             # 16 t-tiles
    QC = 512                  # q-chunk width in attention
    NQC = T // QC             # 4 q-chunks
    VS = D + 1                # 65: V cols + ones col per head

    norm_q = []   # staged normalization closures (pumped 1 per tk)

    from contextlib import ExitStack
    with ExitStack() as ctx:
        consts = ctx.enter_context(tc.tile_pool(name="consts", bufs=1))
        xs_pool = ctx.enter_context(tc.tile_pool(name="xs", bufs=12))
        big = ctx.enter_context(tc.tile_pool(name="big", bufs=1))
        e_pool = ctx.enter_context(tc.tile_pool(name="e", bufs=6))
        ev_pool = ctx.enter_context(tc.tile_pool(name="ev", bufs=3))
        nrm_pool = ctx.enter_context(tc.tile_pool(name="nrm", bufs=4))
        pp = ctx.enter_context(tc.tile_pool(name="pp", bufs=2, space="PSUM"))
        pv_ps = ctx.enter_context(tc.tile_pool(name="pvps", bufs=2, space="PSUM"))
        sa_ps = ctx.enter_context(tc.tile_pool(name="saps", bufs=1, space="PSUM"))
        sb_ps = ctx.enter_context(tc.tile_pool(name="sbps", bufs=1, space="PSUM"))

        # Per-chunk persistent activations: fine-grained tiles so stages
        # pipeline at chunk granularity instead of a hard phase boundary.
        qTc = [big.tile([128, MT, MC], BF16, name=f"qTc{i}", tag=f"qTc{i}")
               for i in range(NMC)]
        kTc = [big.tile([128, MT, MC], BF16, name=f"kTc{i}", tag=f"kTc{i}")
               for i in range(NMC)]
        vc = [big.tile([128, MC // 128, HG * VS], BF16, name=f"vc{i}",
                       tag=f"vc{i}") for i in range(NMC)]
        # out-proj stationary: head pair g2 stacked on partitions
        # (head 2*g2 -> partitions 0-63, head 2*g2+1 -> partitions 64-127)
        oc = [big.tile([128, HG // 2, QC], BF16, name=f"oc{i}", tag=f"oc{i}")
              for i in range(NQC)]

        wq_s = consts.tile([128, KT, CG], BF16, tag="wq")
        wk_s = consts.tile([128, KT, CG], BF16, tag="wk")
        wv_s = consts.tile([128, KT, CG], BF16, tag="wv")
        wo_s = consts.tile([128, HG // 2, C], BF16, tag="wo")
        bq_s = consts.tile([128, MT, 1], F32, tag="bq")
        bk_s = consts.tile([128, MT, 1], F32, tag="bk")
        bv_bc = consts.tile([128, CG], F32, tag="bv")

        def load_x(src, name, eng=None):
            x_t = xs_pool.tile([128, KT, MC], BF16, tag="xs", name=name)
            (eng or nc.sync).dma_start(
                out=x_t, in_=src.rearrange("(kt p) m -> p kt m", p=128))
            return x_t

        def emit_a(mc, xq_t, xk_t):
            """Q/K projection for chunk mc; yields every 4 matmuls."""
            for x_t, w_s, b_s, dstl in ((xq_t, wq_s, bq_s, qTc),
                                        (xk_t, wk_s, bk_s, kTc)):
                for mt in range(MT):
                    ps = pp.tile([128, 512], F32, tag="pp")
                    for kt in range(KT):
                        nc.tensor.matmul(
                            ps[:, :MC],
                            w_s[:, kt, bass.ts(mt, 128)],
                            x_t[:, kt, :],
                            start=(kt == 0), stop=(kt == KT - 1))
                        if kt == 3:
                            yield
                    nc.vector.tensor_scalar_add(
                        dstl[mc][:, mt, :], ps[:, :MC], b_s[:, mt, :])
                    yield

        def emit_b(mc, xv_t):
            """V projection for chunk mc; yields every 8 matmuls (2 subs)."""
            v4 = vc[mc].rearrange("p t (h c) -> p t h c", h=HG)
            nc.vector.memset(v4[:, :, :, D:VS], 1.0)
            for sub in range(MC // 128):
                ps = pp.tile([128, 512], F32, tag="pp")
                for kt in range(KT):
                    nc.tensor.matmul(
                        ps[:, :CG],
                        xv_t[:, kt, bass.ts(sub, 128)],
                        wv_s[:, kt, :],
                        start=(kt == 0), stop=(kt == KT - 1))
                    if kt == 3:
                        yield
                nc.vector.tensor_add(
                    v4[:, sub, :, 0:D],
                    ps[:, :CG].rearrange("p (h c) -> p h c", h=HG),
                    bv_bc.rearrange("p (h c) -> p h c", h=HG))
                yield

        d_ctr = [0]

        def emit_d_unit(qc, tl, cc):
            trows_out = bass.ts(qc * (QC // 128) + tl, 128)
            ps = pp.tile([128, 512], F32, tag="pp")
            for g2 in range(HG // 2):
                nc.tensor.matmul(
                    ps,
                    oc[qc][:, g2, bass.ts(tl, 128)],
                    wo_s[:, g2, bass.ts(cc, 512)],
                    start=(g2 == 0), stop=(g2 == HG // 2 - 1))
            ev = ev_pool.tile([128, 512], F32, tag="ev")
            d_ctr[0] += 1
            if d_ctr[0] % 2 == 0:
                nc.scalar.copy(ev, ps)
            else:
                nc.vector.tensor_copy(ev, ps)
            nc.sync.dma_start(out=yp[trows_out, bass.ts(cc, 512)], in_=ev)

        def c_iter(qc, hp):
            """Attention for head pair hp on q-chunk qc.  Yields after every
            tk so filler matmuls can interleave at fine granularity."""
            po = [pv_ps.tile([128, 512], F32, tag="pv",
                             name=f"po{qc}_{hp}_{i}") for i in range(2)]
            def emit_pv(e_prev, tkp):
                for h01 in range(2):
                    nc.tensor.matmul(
                        po[h01][0:VS, :],
                        vc[tkp // 4][:, tkp % 4,
                                     bass.ds((2 * hp + h01) * VS, VS)],
                        e_prev[:, h01, :],
                        start=(tkp == 0), stop=(tkp == TT - 1))

            pending = []      # issue PV two tks late so the exp latency
            for tk in range(TT):   # never gates the tensor chain
                pool = sa_ps if tk % 2 == 0 else sb_ps
                ps = pool.tile([128, 2, 512], F32,
                               tag="sa" if tk % 2 == 0 else "sb")
                for h01 in range(2):
                    pb = h01 * D
                    nc.tensor.matmul(
                        ps[:, h01, :],
                        kTc[tk // 4][pb:pb + D, hp, bass.ts(tk % 4, 128)],
                        qTc[qc][pb:pb + D, hp, :],
                        start=True, stop=True)
                e_t = e_pool.tile([128, 2, 512], BF16, tag="e")
                if tk % EXP_DVE_EVERY == EXP_DVE_EVERY - 1:
                    nc.vector.tensor_scalar(e_t.bitcast(I16), ps,
                                            A16, B16, ALU.mult, ALU.add)
                else:
                    nc.scalar.activation(e_t, ps, AF.Exp, scale=SCALE)
                pending.append((e_t, tk))
                if len(pending) > 2:
                    emit_pv(*pending.pop(0))
                yield
            for pe in pending:
                emit_pv(*pe)
            yield
            # Evacuate po to SBUF now (frees the pv psum tiles for the next
            # c_iter's PV almost immediately); the rest of the normalization
            # is staged into norm_q, pumped one op per tk during the next
            # c_iter so the serial chain never blocks the DVE-exp stream.
            pocs = []
            for h01 in range(2):
                poc = nrm_pool.tile([128, 512], F32, tag="poc",
                                    name=f"poc{qc}_{hp}_{h01}")
                nc.vector.tensor_copy(poc[0:VS, :], po[h01][0:VS, :])
                pocs.append(poc)

            def stage(h01):
                poc = pocs[h01]
                den_s = nrm_pool.tile([1, 2, 512], F32, tag="dens")
                rec_b = nrm_pool.tile([64, 512], F32, tag="recb")
                yield lambda: nc.vector.tensor_mul(
                    den_s[:, 0, :], poc[D:D + 1, :], ones_r[D:D + 1, :])
                yield lambda: nc.vector.reciprocal_approx_fast(
                    den_s[:, 1, :], den_s[:, 0, :])
                yield lambda: nc.gpsimd.partition_broadcast(
                    rec_b, den_s[:, 1, :])
                yield lambda: nc.vector.tensor_mul(
                    oc[qc][64 * h01:64 * h01 + 64, hp, :],
                    poc[0:D, :], rec_b)

            # interleave the two heads' chains so the serial latency halves
            for a, b in zip(stage(0), stage(1)):
                norm_q.append(a)
                norm_q.append(b)

        # ---- emission schedule (software pipeline) --------------------
        # Critical-path DMAs first (Q-proj inputs), everything prefetched up
        # front so proj matmuls never head-of-line block the tensor queue.
        nc.sync.dma_start(out=wq_s, in_=wq.rearrange("(kt p) c -> p kt c", p=128))
        xq_ts = {0: load_x(xqT[:, bass.ts(0, MC)], "xq0", eng=nc.scalar)}
        nc.sync.dma_start(out=wk_s, in_=wk.rearrange("(kt p) c -> p kt c", p=128))
        xk_ts = {0: load_x(xkT[:, bass.ts(0, MC)], "xk0", eng=nc.scalar)}
        nc.sync.dma_start(
            out=bq_s, in_=bq.rearrange("(mt p) -> p mt", p=128).unsqueeze(2))
        nc.sync.dma_start(
            out=bk_s, in_=bk.rearrange("(mt p) -> p mt", p=128).unsqueeze(2))
        nc.sync.dma_start(out=wv_s, in_=wv.rearrange("(kt p) c -> p kt c", p=128))
        nc.sync.dma_start(
            out=bv_bc,
            in_=bass.AP(tensor=bv.tensor, offset=bv.offset,
                        ap=[[0, 128]] + list(bv.ap)))
        xv_ts = {0: load_x(xvT[:, bass.ts(0, MC)], "xv0")}
        # touch Exp once so the ACT table loads during the DMA-bound head
        warm = consts.tile([128, 1], F32, tag="warm")
        nc.vector.memset(warm[0:1, :], 0.0)
        nc.scalar.activation(warm[0:1, :], warm[0:1, :], AF.Exp)
        ones_r = consts.tile([128, 512], F32, tag="onesr")
        nc.vector.memset(ones_r[D:D + 1, :], 1.0)
        # PE warm-up: dummy matmuls during the DMA head ramp the p-state
        # (PE reaches max clock after ~3us of continuous execution).
        wx = consts.tile([128, 512], BF16, tag="wx")
        nc.vector.memset(wx[:, 0:8], 0.0)
        psd = pp.tile([128, 512], F32, tag="pp", name="warmps")
        for i in range(16):
            nc.tensor.matmul(psd[0:8, :], wx[:, 0:8], wx,
                             start=(i == 0), stop=(i == 15))
        for mc in range(1, NMC):
            xq_ts[mc] = load_x(xqT[:, bass.ts(mc, MC)], f"xq{mc}")
            xk_ts[mc] = load_x(xkT[:, bass.ts(mc, MC)], f"xk{mc}")
            xv_ts[mc] = load_x(xvT[:, bass.ts(mc, MC)], f"xv{mc}")
        nc.sync.dma_start(out=wo_s, in_=wo.rearrange("(g p) c -> p g c", p=128))

        # chunk 0 proj fully, before attention can start
        for _ in emit_a(0, xq_ts[0], xk_ts[0]):
            pass
        for _ in emit_b(0, xv_ts[0]):
            pass

        # filler stream: proj chunks 1-3 (quanta), then out-proj d-units
        def filler_stream():
            for mc in range(1, NMC):
                yield from emit_a(mc, xq_ts[mc], xk_ts[mc])
                yield from emit_b(mc, xv_ts[mc])

        fillers = filler_stream()
        proj_left = 3 * 16           # filler quanta in proj phase
        d_queue = []                 # (qc, tl, cc) units, 2 MMs each

        def pump(n):
            nonlocal proj_left
            for _ in range(n):
                if norm_q:
                    norm_q.pop(0)()
                elif next(fillers, StopIteration) is not StopIteration:
                    proj_left -= 1
                elif d_queue:
                    emit_d_unit(*d_queue.pop(0))
                else:
                    break

        for qc in range(NQC):
            for hp in range(HG // 2):
                gen = c_iter(qc, hp)
                if hp == 0 and qc > 0:       # qc-1 fully staged by now
                    d_queue += [(qc - 1, tl, cc)
                                for tl in range(4) for cc in range(2)]
                for _ in range(TT):
                    next(gen)
                    # proj phase: 4 quanta/tk keeps chunk mc fully emitted
                    # before attention reaches tk=4*mc (in-order queues would
                    # otherwise head-of-line deadlock); then 1 op/tk.
                    pump(4 if proj_left > 0 else 1)
                for _ in gen:                # final PV + po evac + staging
                    pass
        pump(10**6)
        d_queue += [(NQC - 1, tl, cc) for tl in range(4) for cc in range(2)]
        for u in d_queue:
            emit_d_unit(*u)


_NC_CACHE = None


def _get_program():
    global _NC_CACHE
    if _NC_CACHE is None:
        _NC_CACHE = build_mha_program()
    return _NC_CACHE


def make_in_maps(query, key, value, Wq, bq, Wk, bk, Wv, bv, Wo):
    q = np.asarray(query, np.float32).reshape(B, T, C)
    k = np.asarray(key, np.float32).reshape(B, T, C)
    v = np.asarray(value, np.float32).reshape(B, T, C)
    xT = {n: [np.ascontiguousarray(a[b].T).astype(ml_dtypes.bfloat16)
              for b in range(B)]
          for n, a in (("q", q), ("k", k), ("v", v))}
    in_maps = []
    for c in range(N_CORES):
        b, g = divmod(c, GROUPS)
        sl = slice(g * CG, (g + 1) * CG)
        in_maps.append({
            "xqT": xT["q"][b], "xkT": xT["k"][b], "xvT": xT["v"][b],
            "wq": np.ascontiguousarray(np.asarray(Wq, np.float32)[:, sl]).astype(ml_dtypes.bfloat16),
            "wk": np.ascontiguousarray(np.asarray(Wk, np.float32)[:, sl]).astype(ml_dtypes.bfloat16),
            "wv": np.ascontiguousarray(np.asarray(Wv, np.float32)[:, sl]).astype(ml_dtypes.bfloat16),
            "bq": np.ascontiguousarray(np.asarray(bq, np.float32)[sl]),
            "bk": np.ascontiguousarray(np.asarray(bk, np.float32)[sl]),
            "bv": np.ascontiguousarray(np.asarray(bv, np.float32)[sl]),
            "wo": np.ascontiguousarray(np.asarray(Wo, np.float32)[sl, :]).astype(ml_dtypes.bfloat16),
        })
    return in_maps


def assemble_output(results, bo):
    y = np.zeros((B, T, C), np.float32)
    for c, res in enumerate(results):
        y[c // GROUPS] += res["yp"]
    y += np.asarray(bo, np.float32)
    return y


def kernel(query, key, value, Wq, bq, Wk, bk, Wv, bv, Wo, bo):
    nc = _get_program()
    in_maps = make_in_maps(query, key, value, Wq, bq, Wk, bk, Wv, bv, Wo)
    res = run_bass_kernel_spmd(nc, in_maps, list(range(N_CORES)))
    return assemble_output(res.results, bo)
